# revision 43
# baseline (speedup 1.0000x reference)
"""Trainium2 Bass kernel for nn_DualAttention.

Math (per batch b, data-parallel over 8 cores):
  E   = encoder_hidden.reshape(B*S, He)[b*S:(b+1)*S]      # contiguous slab
  X   = tanh(E @ W1 + b1)        [S, D]
  F   = tanh(input_z[b] @ W3 + b3)  [S, D]
  O2  = tanh(output[b] @ W2 + b2)   [T, D]
  O3  = tanh(output[b] @ W4 + b4)   [T, D]
  g[s,t,d] = X[s,d]*O2[t,d] + F[s,d]*O3[t,d]
  gamma    = softmax_s(g)        (== alpha*beta/qn of the reference, exactly)
  contex[t,d] = sum_s gamma[s,t,d] * E[s,d]
  attn[b,t,s,d] = gamma[s,t,d]
  concat = [output, contex]      (assembled host-side)

Layout: d on partitions (4 chunks of 128), s on the free dim.  O2[t,:],
O3[t,:], and 1/Z are then per-partition scalars, so the per-t chain is:
  v   = FT*o3[t]           (1 chunk ScalarE scale-copy, 3 chunks GpSimd TT
                            with a stride-0 free-dim broadcast AP)
  g   = XT*o2[t] + v       (VectorE scalar_tensor_tensor, fused)
  p   = exp(g), Z = sum_s  (ScalarE activation with accum_out)
  gam = p * (1/Z)          (GpSimd TT, stride-0 broadcast)
  pe  = (p*(1/Z))*E, cnum  (VectorE scalar_tensor_tensor with accum_out
                            -> contex, already normalized)
TensorE only does fp32 work at setup (input transposes + the four linears)
plus per-t 128x128 transposes of gamma to [s,d] so attn stores are
contiguous 256KB tiles (PSUM drained by ScalarE copies; DMA cannot read
PSUM here).  Engine balance measured on HW: DVE ~92%, GpSimd ~80%,
ScalarE ~73%, TensorE ~64%; ~237us for all 8 cores in parallel.

Toolchain workaround: this walrus build rejects any instruction carrying
more than one semaphore wait, so _split_waits() post-processes the
scheduled BIR (see its docstring).
"""

import numpy as np
from contextlib import ExitStack

import bass_rust
import concourse.bass as bass
import concourse.tile as tile
from concourse import mybir
from concourse.bass_utils import run_bass_kernel_spmd
from concourse.masks import make_identity

B, S, T, D = 8, 256, 32, 512
HE, FIELD = 512, 128
FP32 = mybir.dt.float32
N_CORES = 8
P = 128
TB = 2          # t-block size for batching reciprocals
V_ACT_CHUNKS = 3  # how many of the 4 v-chunks run on ScalarE (rest on VectorE)

TRACE = False
LAST_RESULTS = None

Tanh = mybir.ActivationFunctionType.Tanh
Exp = mybir.ActivationFunctionType.Exp
Copy = mybir.ActivationFunctionType.Copy


def ts(i, n=P):
    return slice(i * n, (i + 1) * n)


def _split_waits(nc, max_waits=1):
    """Walrus in this toolchain rejects instructions carrying more than one
    semaphore wait ("Too many sync wait commands").  Tile emits multi-wait
    instructions freely, so split the excess off onto single-wait
    EventSemaphore instructions inserted just before, on the same engine
    queue — semantically identical (all waits still execute, in order,
    before the instruction)."""
    n_split = 0
    for f in nc.m.functions:
        for bb in f.blocks:
            insl = bb.instructions
            i = 0
            while i < len(insl):
                ins = insl[i]
                si = getattr(ins, "sync_info", None)
                if si is not None and len(si.on_wait) > max_waits:
                    waits = list(si.on_wait)
                    keep = waits[-max_waits:]
                    hoist = waits[:-max_waits]
                    for j, w in enumerate(hoist):
                        ev = mybir.InstEventSemaphore(
                            name=f"{ins.name}-w{j}",
                            engine=ins.engine,
                            ins=[],
                            outs=[],
                            sync_info=bass_rust.SyncInfo(on_wait=[w], on_update=[]),
                        )
                        insl.insert(i, ev)
                        i += 1
                    ins.sync_info = bass_rust.SyncInfo(
                        on_wait=keep, on_update=list(si.on_update)
                    )
                    n_split += 1
                i += 1
    return n_split


def _build_program():
    nc = bass.Bass(trn_type="TRN2", target_bir_lowering=False)

    E_d = nc.dram_tensor("E", (S, HE), FP32, kind="ExternalInput")
    Zin_d = nc.dram_tensor("Zin", (S, FIELD), FP32, kind="ExternalInput")
    Ob_d = nc.dram_tensor("OutB", (T, D), FP32, kind="ExternalInput")
    W1_d = nc.dram_tensor("W1", (HE, D), FP32, kind="ExternalInput")
    W2_d = nc.dram_tensor("W2", (D, D), FP32, kind="ExternalInput")
    W3_d = nc.dram_tensor("W3", (FIELD, D), FP32, kind="ExternalInput")
    W4_d = nc.dram_tensor("W4", (D, D), FP32, kind="ExternalInput")
    b1_d = nc.dram_tensor("b1", (1, D), FP32, kind="ExternalInput")
    b2_d = nc.dram_tensor("b2", (1, D), FP32, kind="ExternalInput")
    b3_d = nc.dram_tensor("b3", (1, D), FP32, kind="ExternalInput")
    b4_d = nc.dram_tensor("b4", (1, D), FP32, kind="ExternalInput")

    attn_d = nc.dram_tensor("attn", (T, S, D), FP32, kind="ExternalOutput")
    ctx_d = nc.dram_tensor("contex", (T, D), FP32, kind="ExternalOutput")

    with tile.TileContext(nc) as tc, ExitStack() as ctx:
        singles = ctx.enter_context(tc.tile_pool(name="singles", bufs=1))
        work = ctx.enter_context(tc.tile_pool(name="work", bufs=4))
        pbuf = ctx.enter_context(tc.tile_pool(name="pbuf", bufs=2 * TB + 4))
        store = ctx.enter_context(tc.tile_pool(name="store", bufs=6))

        ident = singles.tile([P, P], FP32)
        make_identity(nc, ident)
        ones = singles.tile([P, 2 * P], FP32)
        nc.vector.memset(ones, 1.0)

        # ---- loads ----
        E_sb = singles.tile([P, 2, HE], FP32)
        Zin_sb = singles.tile([P, 2, FIELD], FP32)
        for sh in range(2):
            nc.sync.dma_start(out=E_sb[:, sh, :], in_=E_d[ts(sh), :])
            nc.sync.dma_start(out=Zin_sb[:, sh, :], in_=Zin_d[ts(sh), :])
        Ob_sb = singles.tile([T, D], FP32)
        nc.sync.dma_start(out=Ob_sb, in_=Ob_d[:, :])

        # biases as d-partitioned columns [128, 4] (chunk c in column c) so the
        # tanh activations can take them as per-partition bias directly; one
        # strided DMA each, early on the sync queue (DMA issue costs ~650ns
        # per dma_start on the issuing engine queue)
        b_sb = {}
        for name, bd in (("b1", b1_d), ("b2", b2_d), ("b3", b3_d), ("b4", b4_d)):
            b_sb[name] = singles.tile([P, 4], FP32, name=f"bias_{name}")
            nc.sync.dma_start(
                out=b_sb[name],
                in_=bd[0:1, :].rearrange("a (c p) -> p (a c)", c=4),
            )
        # weights on the gpsimd SWDGE queue; W3 first (feeds FT -> first v)
        W1_sb = singles.tile([P, 4, D], FP32)
        W2_sb = singles.tile([P, 4, D], FP32)
        W4_sb = singles.tile([P, 4, D], FP32)
        W3_sb = singles.tile([P, D], FP32)
        nc.gpsimd.dma_start(out=W3_sb, in_=W3_d[:, :])
        for kc in range(4):
            nc.gpsimd.dma_start(out=W1_sb[:, kc, :], in_=W1_d[ts(kc), :])
        for kc in range(4):
            nc.gpsimd.dma_start(out=W4_sb[:, kc, :], in_=W4_d[ts(kc), :])
        for kc in range(4):
            nc.gpsimd.dma_start(out=W2_sb[:, kc, :], in_=W2_d[ts(kc), :])

        # ---- setup: transposes + linears (all d-on-partitions) ----
        # ET[p, kc, s] = E[s, kc*128+p]  — serves as matmul rhs (K=He) and as
        # the d-partitioned E for the contex reduction.
        ET = singles.tile([P, 4, S], FP32)
        ZT = singles.tile([P, S], FP32)
        OT = singles.tile([P, P], FP32)    # OT[p, kc*32+t] = OutB[t, kc*128+p]
        XT = singles.tile([P, 4, S], FP32)
        FT = singles.tile([P, 4, S], FP32)
        o2col = singles.tile([P, 4, T], FP32)
        o3col = singles.tile([P, 4, T], FP32)

        with tc.tile_pool(name="ps_setup", bufs=3, space="PSUM") as psA:
            for kc in range(4):
                pt = psA.tile([P, S], FP32, tag="mm")
                for sh in range(2):
                    nc.tensor.transpose(pt[:, ts(sh)], E_sb[:, sh, ts(kc)], ident)
                nc.scalar.copy(out=ET[:, kc, :], in_=pt)
            ptz = psA.tile([P, S], FP32, tag="mm")
            for sh in range(2):
                nc.tensor.transpose(ptz[:, ts(sh)], Zin_sb[:, sh, :], ident)
            nc.scalar.copy(out=ZT, in_=ptz)
            pto = psA.tile([P, S], FP32, tag="mm")
            for kc in range(4):
                nc.tensor.transpose(
                    pto[:, kc * T:(kc + 1) * T], Ob_sb[:, ts(kc)], ident[0:T, 0:T]
                )
            nc.scalar.copy(out=OT, in_=pto[:, 0:P])

            # Linears, emitted chunk-major so the t-loop's chunk-0 deps
            # (FT/o3col/XT/o2col at mc=0) resolve as early as possible.
            for mc in range(4):
                # FT = tanh(W3^T @ Zin^T + b3)
                pm = psA.tile([P, S], FP32, tag="mm")
                nc.tensor.matmul(pm, W3_sb[:, ts(mc)], ZT, start=True, stop=True)
                nc.scalar.activation(
                    FT[:, mc, :], pm, Tanh, bias=b_sb["b3"][:, mc:mc + 1]
                )
                # o3col = tanh(W4^T @ OutB^T + b4)[d, t]
                pm = psA.tile([P, S], FP32, tag="mm")
                for kc in range(4):
                    nc.tensor.matmul(
                        pm[:, 0:T], W4_sb[:, kc, ts(mc)],
                        OT[:, kc * T:(kc + 1) * T],
                        start=(kc == 0), stop=(kc == 3),
                    )
                nc.scalar.activation(
                    o3col[:, mc, :], pm[:, 0:T], Tanh,
                    bias=b_sb["b4"][:, mc:mc + 1],
                )
                # XT = tanh(W1^T @ E^T + b1)
                pm = psA.tile([P, S], FP32, tag="mm")
                for kc in range(4):
                    nc.tensor.matmul(
                        pm, W1_sb[:, kc, ts(mc)], ET[:, kc, :],
                        start=(kc == 0), stop=(kc == 3),
                    )
                nc.scalar.activation(
                    XT[:, mc, :], pm, Tanh, bias=b_sb["b1"][:, mc:mc + 1]
                )
                # o2col = tanh(W2^T @ OutB^T + b2)[d, t]
                pm = psA.tile([P, S], FP32, tag="mm")
                for kc in range(4):
                    nc.tensor.matmul(
                        pm[:, 0:T], W2_sb[:, kc, ts(mc)],
                        OT[:, kc * T:(kc + 1) * T],
                        start=(kc == 0), stop=(kc == 3),
                    )
                nc.scalar.activation(
                    o2col[:, mc, :], pm[:, 0:T], Tanh,
                    bias=b_sb["b2"][:, mc:mc + 1],
                )

        zpool = ctx.enter_context(tc.tile_pool(name="zpool", bufs=3))
        rows_pool = ctx.enter_context(tc.tile_pool(name="rows", bufs=4))

        # ---- main t-loop, in blocks of TB ----
        with (
            tc.tile_pool(name="ps_t", bufs=3, space="PSUM") as psT,
            tc.tile_pool(name="ps_r", bufs=2, space="PSUM") as psR,
        ):
            for tb in range(T // TB):
                p_tiles = []
                Zblk = zpool.tile([P, 4, TB], FP32, tag="z")
                rzblk = zpool.tile([P, 4, TB], FP32, tag="rz")
                for tj in range(TB):
                    t = tb * TB + tj
                    # v = FT * o3[t]: 1 chunk on ScalarE, 3 on GpSimd (stride-0
                    # free-dim broadcast of the per-partition scalar)
                    v = work.tile([P, 4, S], FP32, tag="v")
                    nc.scalar.activation(
                        v[:, 0, :], FT[:, 0, :], Copy, scale=o3col[:, 0, t:t + 1]
                    )
                    for c in range(1, 4):
                        sl = o3col[:, c, t:t + 1]
                        o3b = bass.AP(
                            tensor=sl.tensor, offset=sl.offset, ap=[sl.ap[0], [0, S]]
                        )
                        nc.gpsimd.tensor_mul(v[:, c, :], FT[:, c, :], o3b)
                    # g = XT * o2[t] + v, fused
                    g = work.tile([P, 4, S], FP32, tag="g")
                    for c in range(4):
                        nc.vector.scalar_tensor_tensor(
                            out=g[:, c, :], in0=XT[:, c, :],
                            scalar=o2col[:, c, t:t + 1], in1=v[:, c, :],
                            op0=mybir.AluOpType.mult, op1=mybir.AluOpType.add,
                        )
                    p = pbuf.tile([P, 4, S], FP32, tag="p")
                    for c in range(4):
                        nc.scalar.activation(
                            p[:, c, :], g[:, c, :], Exp,
                            accum_out=Zblk[:, c, tj:tj + 1],
                        )
                    p_tiles.append(p)
                # 1/Z for the whole block in one wide op
                nc.vector.reciprocal(rzblk, Zblk)

                for tj in range(TB):
                    t = tb * TB + tj
                    p = p_tiles[tj]
                    # gamma = p * (1/Z), broadcast along s via stride-0 AP;
                    # runs on the otherwise-idle GpSimd engine
                    gam = work.tile([P, 4, S], FP32, tag="gam", bufs=6)
                    nc.scalar.activation(
                        gam[:, 0, :], p[:, 0, :], Copy, scale=rzblk[:, 0, tj:tj + 1]
                    )
                    for c in range(1, 4):
                        sl = rzblk[:, c, tj:tj + 1]
                        rzb = bass.AP(
                            tensor=sl.tensor, offset=sl.offset, ap=[sl.ap[0], [0, S]]
                        )
                        nc.gpsimd.tensor_mul(gam[:, c, :], p[:, c, :], rzb)
                    # transpose [d, s] -> [s, d] and store contiguously
                    psx = psT.tile([P, 2, D], FP32, tag="pst")
                    for sh in range(2):
                        for c in range(4):
                            nc.tensor.transpose(
                                psx[:, sh, ts(c)], gam[:, c, ts(sh)], ident
                            )
                    gsb = store.tile([P, 2, D], FP32, tag="gsb")
                    nc.scalar.copy(out=gsb, in_=psx)
                    for sh in range(2):
                        nc.sync.dma_start(out=attn_d[t, ts(sh), :], in_=gsb[:, sh, :])
                    # contex[t, :] = sum_s gammaT*E: one wide DVE multiply on the
                    # transposed tile, then a K=128x2 ones-matmul on TensorE
                    peT = work.tile([P, 2, D], FP32, tag="peT", bufs=3)
                    nc.vector.tensor_mul(peT, gsb, E_sb)
                    crow = psR.tile([1, D], FP32, tag="crow")
                    nc.tensor.matmul(crow, ones[:, 0:1], peT[:, 0, :], start=True, stop=False)
                    nc.tensor.matmul(crow, ones[:, 0:1], peT[:, 1, :], start=False, stop=True)
                    ctxrow = rows_pool.tile([1, D], FP32, tag="ctxrow")
                    nc.scalar.copy(out=ctxrow, in_=crow)
                    nc.sync.dma_start(out=ctx_d[t:t + 1, :], in_=ctxrow)


    _split_waits(nc)
    return nc


_PROGRAM = None


def kernel(output, encoder_hidden, input_z, W1, b1, W2, b2, W3, b3, W4, b4):
    global _PROGRAM, LAST_RESULTS
    output = np.ascontiguousarray(np.asarray(output, dtype=np.float32))
    encoder_hidden = np.ascontiguousarray(np.asarray(encoder_hidden, dtype=np.float32))
    input_z = np.ascontiguousarray(np.asarray(input_z, dtype=np.float32))

    E2d = encoder_hidden.reshape(B * S, HE)
    common = {
        "W1": np.ascontiguousarray(W1), "W2": np.ascontiguousarray(W2),
        "W3": np.ascontiguousarray(W3), "W4": np.ascontiguousarray(W4),
        "b1": np.ascontiguousarray(np.asarray(b1).reshape(1, D)),
        "b2": np.ascontiguousarray(np.asarray(b2).reshape(1, D)),
        "b3": np.ascontiguousarray(np.asarray(b3).reshape(1, D)),
        "b4": np.ascontiguousarray(np.asarray(b4).reshape(1, D)),
    }
    in_maps = []
    for b in range(B):
        m = dict(common)
        m["E"] = np.ascontiguousarray(E2d[b * S:(b + 1) * S])
        m["Zin"] = np.ascontiguousarray(input_z[b])
        m["OutB"] = np.ascontiguousarray(output[b])
        in_maps.append(m)

    if _PROGRAM is None:
        _PROGRAM = _build_program()

    res = run_bass_kernel_spmd(
        _PROGRAM, in_maps, core_ids=list(range(N_CORES)), trace=TRACE,
    )
    LAST_RESULTS = res

    attn = np.stack([res.results[b]["attn"] for b in range(B)], axis=0)
    contex = np.stack([res.results[b]["contex"] for b in range(B)], axis=0)
    concat = np.concatenate([output, contex], axis=-1)
    return concat, attn


# revision 44
# speedup vs baseline: 1.3301x; 1.3301x over previous
"""Trainium2 Bass kernel for nn_DualAttention.

Math (per batch b, data-parallel over 8 cores):
  E   = encoder_hidden.reshape(B*S, He)[b*S:(b+1)*S]      # contiguous slab
  X   = tanh(E @ W1 + b1)        [S, D]
  F   = tanh(input_z[b] @ W3 + b3)  [S, D]
  O2  = tanh(output[b] @ W2 + b2)   [T, D]
  O3  = tanh(output[b] @ W4 + b4)   [T, D]
  g[s,t,d] = X[s,d]*O2[t,d] + F[s,d]*O3[t,d]
  gamma    = softmax_s(g)        (== alpha*beta/qn of the reference, exactly)
  contex[t,d] = sum_s gamma[s,t,d] * E[s,d]
  attn[b,t,s,d] = gamma[s,t,d]
  concat = [output, contex]      (assembled host-side)

Layout: d on partitions (4 chunks of 128), s on the free dim.  O2[t,:],
O3[t,:], and 1/Z are then per-partition scalars, so the per-t chain is:
  v   = FT*o3[t]           (1 chunk ScalarE scale-copy, 3 chunks GpSimd TT
                            with a stride-0 free-dim broadcast AP)
  g   = XT*o2[t] + v       (VectorE scalar_tensor_tensor, fused)
  p   = exp(g), Z = sum_s  (ScalarE activation with accum_out)
  gam = p * (1/Z)          (GpSimd TT, stride-0 broadcast)
  pe  = (p*(1/Z))*E, cnum  (VectorE scalar_tensor_tensor with accum_out
                            -> contex, already normalized)
TensorE only does fp32 work at setup (input transposes + the four linears)
plus per-t 128x128 transposes of gamma to [s,d] so attn stores are
contiguous 256KB tiles (PSUM drained by ScalarE copies; DMA cannot read
PSUM here).  Engine balance measured on HW: DVE ~92%, GpSimd ~80%,
ScalarE ~73%, TensorE ~64%; ~237us for all 8 cores in parallel.

Toolchain workaround: this walrus build rejects any instruction carrying
more than one semaphore wait, so _split_waits() post-processes the
scheduled BIR (see its docstring).
"""

import numpy as np
from contextlib import ExitStack

import bass_rust
import concourse.bass as bass
import concourse.tile as tile
from concourse import mybir
from concourse.bass_utils import run_bass_kernel_spmd
from concourse.masks import make_identity

B, S, T, D = 8, 256, 32, 512
HE, FIELD = 512, 128
FP32 = mybir.dt.float32
N_CORES = 8
P = 128
TB = 2          # t-block size for batching reciprocals
V_ACT_CHUNKS = 3  # how many of the 4 v-chunks run on ScalarE (rest on VectorE)

TRACE = False
LAST_RESULTS = None

Tanh = mybir.ActivationFunctionType.Tanh
Exp = mybir.ActivationFunctionType.Exp
Copy = mybir.ActivationFunctionType.Copy


def ts(i, n=P):
    return slice(i * n, (i + 1) * n)


def _split_waits(nc, max_waits=1):
    """Walrus in this toolchain rejects instructions carrying more than one
    semaphore wait ("Too many sync wait commands").  Tile emits multi-wait
    instructions freely, so split the excess off onto single-wait
    EventSemaphore instructions inserted just before, on the same engine
    queue — semantically identical (all waits still execute, in order,
    before the instruction)."""
    n_split = 0
    for f in nc.m.functions:
        for bb in f.blocks:
            insl = bb.instructions
            i = 0
            while i < len(insl):
                ins = insl[i]
                si = getattr(ins, "sync_info", None)
                if si is not None and len(si.on_wait) > max_waits:
                    waits = list(si.on_wait)
                    keep = waits[-max_waits:]
                    hoist = waits[:-max_waits]
                    for j, w in enumerate(hoist):
                        ev = mybir.InstEventSemaphore(
                            name=f"{ins.name}-w{j}",
                            engine=ins.engine,
                            ins=[],
                            outs=[],
                            sync_info=bass_rust.SyncInfo(on_wait=[w], on_update=[]),
                        )
                        insl.insert(i, ev)
                        i += 1
                    ins.sync_info = bass_rust.SyncInfo(
                        on_wait=keep, on_update=list(si.on_update)
                    )
                    n_split += 1
                i += 1
    return n_split


def _build_program():
    nc = bass.Bass(trn_type="TRN2", target_bir_lowering=False)

    E_d = nc.dram_tensor("E", (S, HE), FP32, kind="ExternalInput")
    Zin_d = nc.dram_tensor("Zin", (S, FIELD), FP32, kind="ExternalInput")
    Ob_d = nc.dram_tensor("OutB", (T, D), FP32, kind="ExternalInput")
    W1_d = nc.dram_tensor("W1", (HE, D), FP32, kind="ExternalInput")
    W2_d = nc.dram_tensor("W2", (D, D), FP32, kind="ExternalInput")
    W3_d = nc.dram_tensor("W3", (FIELD, D), FP32, kind="ExternalInput")
    W4_d = nc.dram_tensor("W4", (D, D), FP32, kind="ExternalInput")
    b1_d = nc.dram_tensor("b1", (1, D), FP32, kind="ExternalInput")
    b2_d = nc.dram_tensor("b2", (1, D), FP32, kind="ExternalInput")
    b3_d = nc.dram_tensor("b3", (1, D), FP32, kind="ExternalInput")
    b4_d = nc.dram_tensor("b4", (1, D), FP32, kind="ExternalInput")

    attn_d = nc.dram_tensor("attn", (T, S, D), FP32, kind="ExternalOutput")
    ctx_d = nc.dram_tensor("contex", (T, D), FP32, kind="ExternalOutput")

    with tile.TileContext(nc) as tc, ExitStack() as ctx:
        singles = ctx.enter_context(tc.tile_pool(name="singles", bufs=1))
        work = ctx.enter_context(tc.tile_pool(name="work", bufs=4))
        pbuf = ctx.enter_context(tc.tile_pool(name="pbuf", bufs=2 * TB + 4))
        store = ctx.enter_context(tc.tile_pool(name="store", bufs=6))

        ident = singles.tile([P, P], FP32)
        make_identity(nc, ident)
        ones = singles.tile([P, 2 * P], FP32)
        nc.vector.memset(ones, 1.0)

        # ---- loads ----
        E_sb = singles.tile([P, 2, HE], FP32)
        Zin_sb = singles.tile([P, 2, FIELD], FP32)
        for sh in range(2):
            nc.sync.dma_start(out=E_sb[:, sh, :], in_=E_d[ts(sh), :])
            nc.sync.dma_start(out=Zin_sb[:, sh, :], in_=Zin_d[ts(sh), :])
        Ob_sb = singles.tile([T, D], FP32)
        nc.sync.dma_start(out=Ob_sb, in_=Ob_d[:, :])

        # biases as d-partitioned columns [128, 4] (chunk c in column c) so the
        # tanh activations can take them as per-partition bias directly; one
        # strided DMA each, early on the sync queue (DMA issue costs ~650ns
        # per dma_start on the issuing engine queue)
        b_sb = {}
        for name, bd in (("b1", b1_d), ("b2", b2_d), ("b3", b3_d), ("b4", b4_d)):
            b_sb[name] = singles.tile([P, 4], FP32, name=f"bias_{name}")
            nc.sync.dma_start(
                out=b_sb[name],
                in_=bd[0:1, :].rearrange("a (c p) -> p (a c)", c=4),
            )
        # weights on the gpsimd SWDGE queue; W3 first (feeds FT -> first v)
        W1_sb = singles.tile([P, 4, D], FP32)
        W2_sb = singles.tile([P, 4, D], FP32)
        W4_sb = singles.tile([P, 4, D], FP32)
        W3_sb = singles.tile([P, D], FP32)
        nc.gpsimd.dma_start(out=W3_sb, in_=W3_d[:, :])
        for kc in range(4):
            nc.gpsimd.dma_start(out=W1_sb[:, kc, :], in_=W1_d[ts(kc), :])
        for kc in range(4):
            nc.gpsimd.dma_start(out=W4_sb[:, kc, :], in_=W4_d[ts(kc), :])
        for kc in range(4):
            nc.gpsimd.dma_start(out=W2_sb[:, kc, :], in_=W2_d[ts(kc), :])

        # ---- setup: transposes + linears (all d-on-partitions) ----
        # ET[p, kc, s] = E[s, kc*128+p]  — serves as matmul rhs (K=He) and as
        # the d-partitioned E for the contex reduction.
        ET = singles.tile([P, 4, S], FP32)
        ZT = singles.tile([P, S], FP32)
        OT = singles.tile([P, P], FP32)    # OT[p, kc*32+t] = OutB[t, kc*128+p]
        XT = singles.tile([P, 4, S], FP32)
        FT = singles.tile([P, 4, S], FP32)
        o2col = singles.tile([P, 4, T], FP32)
        o3col = singles.tile([P, 4, T], FP32)

        with tc.tile_pool(name="ps_setup", bufs=3, space="PSUM") as psA:
            for kc in range(4):
                pt = psA.tile([P, S], FP32, tag="mm")
                for sh in range(2):
                    nc.tensor.transpose(pt[:, ts(sh)], E_sb[:, sh, ts(kc)], ident)
                nc.scalar.copy(out=ET[:, kc, :], in_=pt)
            ptz = psA.tile([P, S], FP32, tag="mm")
            for sh in range(2):
                nc.tensor.transpose(ptz[:, ts(sh)], Zin_sb[:, sh, :], ident)
            nc.scalar.copy(out=ZT, in_=ptz)
            pto = psA.tile([P, S], FP32, tag="mm")
            for kc in range(4):
                nc.tensor.transpose(
                    pto[:, kc * T:(kc + 1) * T], Ob_sb[:, ts(kc)], ident[0:T, 0:T]
                )
            nc.scalar.copy(out=OT, in_=pto[:, 0:P])

            # Linears, emitted chunk-major so the t-loop's chunk-0 deps
            # (FT/o3col/XT/o2col at mc=0) resolve as early as possible.
            for mc in range(4):
                # FT = tanh(W3^T @ Zin^T + b3)
                pm = psA.tile([P, S], FP32, tag="mm")
                nc.tensor.matmul(pm, W3_sb[:, ts(mc)], ZT, start=True, stop=True)
                nc.scalar.activation(
                    FT[:, mc, :], pm, Tanh, bias=b_sb["b3"][:, mc:mc + 1]
                )
                # o3col = tanh(W4^T @ OutB^T + b4)[d, t]
                pm = psA.tile([P, S], FP32, tag="mm")
                for kc in range(4):
                    nc.tensor.matmul(
                        pm[:, 0:T], W4_sb[:, kc, ts(mc)],
                        OT[:, kc * T:(kc + 1) * T],
                        start=(kc == 0), stop=(kc == 3),
                    )
                nc.scalar.activation(
                    o3col[:, mc, :], pm[:, 0:T], Tanh,
                    bias=b_sb["b4"][:, mc:mc + 1],
                )
                # XT = tanh(W1^T @ E^T + b1)
                pm = psA.tile([P, S], FP32, tag="mm")
                for kc in range(4):
                    nc.tensor.matmul(
                        pm, W1_sb[:, kc, ts(mc)], ET[:, kc, :],
                        start=(kc == 0), stop=(kc == 3),
                    )
                nc.scalar.activation(
                    XT[:, mc, :], pm, Tanh, bias=b_sb["b1"][:, mc:mc + 1]
                )
                # o2col = tanh(W2^T @ OutB^T + b2)[d, t]
                pm = psA.tile([P, S], FP32, tag="mm")
                for kc in range(4):
                    nc.tensor.matmul(
                        pm[:, 0:T], W2_sb[:, kc, ts(mc)],
                        OT[:, kc * T:(kc + 1) * T],
                        start=(kc == 0), stop=(kc == 3),
                    )
                nc.scalar.activation(
                    o2col[:, mc, :], pm[:, 0:T], Tanh,
                    bias=b_sb["b2"][:, mc:mc + 1],
                )

        cnum = singles.tile([P, 4, T], FP32)
        zpool = ctx.enter_context(tc.tile_pool(name="zpool", bufs=3))

        # ---- main t-loop, in blocks of TB ----
        with tc.tile_pool(name="ps_t", bufs=4, space="PSUM") as psT:
            for tb in range(T // TB):
                p_tiles = []
                Zblk = zpool.tile([P, 4, TB], FP32, tag="z")
                rzblk = zpool.tile([P, 4, TB], FP32, tag="rz")
                for tj in range(TB):
                    t = tb * TB + tj
                    # v = FT * o3[t]: 1 chunk on ScalarE, 3 on GpSimd (stride-0
                    # free-dim broadcast of the per-partition scalar)
                    v = work.tile([P, 4, S], FP32, tag="v")
                    nc.scalar.activation(
                        v[:, 0, :], FT[:, 0, :], Copy, scale=o3col[:, 0, t:t + 1]
                    )
                    for c in range(1, 4):
                        sl = o3col[:, c, t:t + 1]
                        o3b = bass.AP(
                            tensor=sl.tensor, offset=sl.offset, ap=[sl.ap[0], [0, S]]
                        )
                        nc.gpsimd.tensor_mul(v[:, c, :], FT[:, c, :], o3b)
                    # g = XT * o2[t] + v, fused
                    g = work.tile([P, 4, S], FP32, tag="g")
                    for c in range(4):
                        nc.vector.scalar_tensor_tensor(
                            out=g[:, c, :], in0=XT[:, c, :],
                            scalar=o2col[:, c, t:t + 1], in1=v[:, c, :],
                            op0=mybir.AluOpType.mult, op1=mybir.AluOpType.add,
                        )
                    p = pbuf.tile([P, 4, S], FP32, tag="p")
                    for c in range(4):
                        nc.scalar.activation(
                            p[:, c, :], g[:, c, :], Exp,
                            accum_out=Zblk[:, c, tj:tj + 1],
                        )
                    p_tiles.append(p)
                # 1/Z for the whole block in one wide op
                nc.vector.reciprocal(rzblk, Zblk)

                for tj in range(TB):
                    t = tb * TB + tj
                    p = p_tiles[tj]
                    # gamma = p * (1/Z), broadcast along s via stride-0 AP;
                    # runs on the otherwise-idle GpSimd engine
                    gam = work.tile([P, 4, S], FP32, tag="gam", bufs=6)
                    nc.scalar.activation(
                        gam[:, 0, :], p[:, 0, :], Copy, scale=rzblk[:, 0, tj:tj + 1]
                    )
                    for c in range(1, 4):
                        sl = rzblk[:, c, tj:tj + 1]
                        rzb = bass.AP(
                            tensor=sl.tensor, offset=sl.offset, ap=[sl.ap[0], [0, S]]
                        )
                        nc.gpsimd.tensor_mul(gam[:, c, :], p[:, c, :], rzb)
                    # transpose [d, s] -> [s, d] and store contiguously
                    psx = psT.tile([P, 2, D], FP32, tag="pst")
                    for sh in range(2):
                        for c in range(4):
                            nc.tensor.transpose(
                                psx[:, sh, ts(c)], gam[:, c, ts(sh)], ident
                            )
                    gsb = store.tile([P, 2, D], FP32, tag="gsb")
                    nc.scalar.copy(out=gsb, in_=psx)
                    for sh in range(2):
                        nc.sync.dma_start(out=attn_d[t, ts(sh), :], in_=gsb[:, sh, :])
                    # contex summand: (p * rz) * E summed over s via accum_out
                    pe = work.tile([P, 4, S], FP32, tag="pe", bufs=2)
                    for c in range(4):
                        nc.vector.scalar_tensor_tensor(
                            out=pe[:, c, :],
                            in0=p[:, c, :],
                            scalar=rzblk[:, c, tj:tj + 1],
                            in1=ET[:, c, :],
                            op0=mybir.AluOpType.mult,
                            op1=mybir.AluOpType.mult,
                            accum_out=cnum[:, c, t:t + 1],
                        )

            # cnum[d, t] is already sum_s gamma*E = contex^T; transpose to [t, d]
            psC = psT.tile([T, D], FP32, tag="pst")
            for c in range(4):
                nc.tensor.transpose(psC[:, ts(c)], cnum[:, c, :], ident)
            ctx_sb = singles.tile([T, D], FP32)
            nc.scalar.copy(out=ctx_sb, in_=psC)
            nc.sync.dma_start(out=ctx_d[:, :], in_=ctx_sb)

    _split_waits(nc)
    return nc


_PROGRAM = None


def kernel(output, encoder_hidden, input_z, W1, b1, W2, b2, W3, b3, W4, b4):
    global _PROGRAM, LAST_RESULTS
    output = np.ascontiguousarray(np.asarray(output, dtype=np.float32))
    encoder_hidden = np.ascontiguousarray(np.asarray(encoder_hidden, dtype=np.float32))
    input_z = np.ascontiguousarray(np.asarray(input_z, dtype=np.float32))

    E2d = encoder_hidden.reshape(B * S, HE)
    common = {
        "W1": np.ascontiguousarray(W1), "W2": np.ascontiguousarray(W2),
        "W3": np.ascontiguousarray(W3), "W4": np.ascontiguousarray(W4),
        "b1": np.ascontiguousarray(np.asarray(b1).reshape(1, D)),
        "b2": np.ascontiguousarray(np.asarray(b2).reshape(1, D)),
        "b3": np.ascontiguousarray(np.asarray(b3).reshape(1, D)),
        "b4": np.ascontiguousarray(np.asarray(b4).reshape(1, D)),
    }
    in_maps = []
    for b in range(B):
        m = dict(common)
        m["E"] = np.ascontiguousarray(E2d[b * S:(b + 1) * S])
        m["Zin"] = np.ascontiguousarray(input_z[b])
        m["OutB"] = np.ascontiguousarray(output[b])
        in_maps.append(m)

    if _PROGRAM is None:
        _PROGRAM = _build_program()

    res = run_bass_kernel_spmd(
        _PROGRAM, in_maps, core_ids=list(range(N_CORES)), trace=TRACE,
    )
    LAST_RESULTS = res

    attn = np.stack([res.results[b]["attn"] for b in range(B)], axis=0)
    contex = np.stack([res.results[b]["contex"] for b in range(B)], axis=0)
    concat = np.concatenate([output, contex], axis=-1)
    return concat, attn
